# revision 44
# baseline (speedup 1.0000x reference)
"""Exact top-k (k=32) attention on 8 Trainium2 NeuronCores.

Head-parallel: 16 (batch, head) pairs sharded 2-per-core.  The bass
program is specialized (and cached) on SCH = ceil(max(key_lengths)/128):
chunks of keys beyond the valid region are never touched by any engine.

Per head:

Phase 1 (selection), per 128-query L-tile:
  Scores F[q, s] via the 2-pass bf16-split matmul (hi*hi + partial lo*lo
  in pass A; hi*lo + lo*hi in pass BC; ~1e-4 accurate) into [128, 512]
  PSUM quarters.  Top-32 extraction is two-level: per-128-chunk max8
  directly on PSUM (one pass over the data) -> 8*SCH candidates, then
  4x(max8 + match_replace) over the candidates -> 32nd largest t.
  Cut value t_minus = t - |t|*2^-17 - 1e-30; the 2^-17 margin dominates
  the phase1-vs-phase2 accumulation-order rounding noise (the uniform
  weight scale exp(margin/8) cancels in normalization).  t_minus is
  bf16-triple-split into contraction rows 65..67 of the A operand,
  staged via PE transpose + DMA once per group of 4 tiles.

Phase 2 (apply), per group of 512 queries, per 128-key chunk:
  d'[s, q] = F^T - t_minus recomputed by the same matmul pair
  (bit-identical products), then
     g = Exp(temp*d')   (ACT, bf16)
     S = Sign(d')       (ACT, bf16, in {-1,+1})
     b2 = g*S           (Pool, bf16)
  and ONE PSUM accumulation over chunks:
     av += va^T g + va^T b2   = 2 * sum_selected w * v_aug
  va carries an appended ones-column so av row 64 is 2*Z; the factor 2
  cancels in the normalize (PE transpose + DVE reciprocal).

Failure rows are detected from per-tile flag dumps (t_hat, the 33rd
candidate, and each chunk's 8th candidate) and recomputed exactly on
the host: overflow rows (a 128-chunk held >= 9 of the top-32, flagged
by chunk8th >= t_hat) and margin-band rows (33rd candidate within
2^-16 of t_hat).  ~1-3% of rows with the staged key_lengths.
"""

import numpy as np
import ml_dtypes

N, L, S, H, E, D = 2, 2048, 2048, 8, 64, 64
TOPK = 32
TEMP = 1.0 / np.sqrt(E)
HEADS_PER_CORE = 2
N_CORES = 8
LT = 16          # L tiles of 128
QB = 4           # groups of 4 tiles (512 queries)
NEG = -1e30
NLO = 60         # e-rows of the lo*lo partial correction in pass A

_bf16 = ml_dtypes.bfloat16


def _build_bass(schs):
    """schs: per-head number of 128-key chunks covering the valid keys.

    Each core holds one head of each batch, so the per-head chunk counts
    let every core do (sch0 + sch1) chunk-units instead of 2*max(sch)."""
    import concourse.mybir as mybir
    from concourse import bacc
    from concourse.tile import TileContext
    from concourse.masks import make_identity

    f32 = mybir.dt.float32
    bf16 = mybir.dt.bfloat16

    nc = bacc.Bacc()
    HPC = HEADS_PER_CORE
    schmax = max(schs)

    qa_d = nc.declare_dram_parameter("qa", [HPC, 128, L], bf16, isOutput=False)
    ka_d = nc.declare_dram_parameter("ka", [HPC, 128, S], bf16, isOutput=False)
    qbc_d = nc.declare_dram_parameter("qbc", [HPC, 128, L], bf16, isOutput=False)
    kbc_d = nc.declare_dram_parameter("kbc", [HPC, 128, S], bf16, isOutput=False)
    va_d = nc.declare_dram_parameter("va", [HPC, 128, schmax, D + 1], bf16,
                                     isOutput=False)
    out_d = nc.declare_dram_parameter("out", [HPC, L, D], f32, isOutput=True)
    # per-tile host-fix flags: [t_hat, cand33, chunk 8th-candidates...]
    flg_d = nc.declare_dram_parameter("flg", [HPC, LT, 128, 2 + schmax], f32,
                                      isOutput=True)

    from contextlib import ExitStack
    with TileContext(nc) as tc, ExitStack() as ctx:
        consts = ctx.enter_context(tc.tile_pool(name="consts", bufs=1))
        inpool = ctx.enter_context(tc.tile_pool(name="inputs", bufs=1))
        candp = ctx.enter_context(tc.tile_pool(name="cand", bufs=2))
        small = ctx.enter_context(tc.tile_pool(name="small", bufs=2))
        gs_pool = ctx.enter_context(tc.tile_pool(name="gs", bufs=6))
        opool = ctx.enter_context(tc.tile_pool(name="outbuf", bufs=9))
        ps_f = ctx.enter_context(tc.tile_pool(name="ps_fwd", bufs=3,
                                              space="PSUM"))
        ps_t = ctx.enter_context(tc.tile_pool(name="ps_t", bufs=3,
                                              space="PSUM"))
        ps_av = ctx.enter_context(tc.tile_pool(name="ps_av", bufs=1,
                                               space="PSUM"))
        ps_x = ctx.enter_context(tc.tile_pool(name="ps_x", bufs=1,
                                              space="PSUM"))

        # ---- load all inputs (only the covered key region) ----
        qa = []
        ka = []
        qbc = []
        kbc = []
        va = []
        for hh in range(HPC):
            scov = 128 * schs[hh]
            halves = []
            for s2 in range(2):
                t = inpool.tile([128, L // 2], bf16, tag=f"qa{hh}_{s2}",
                                name=f"qa{hh}_{s2}")
                nc.sync.dma_start(t, qa_d[hh][:, s2 * 1024:(s2 + 1) * 1024])
                halves.append(t)
            qa.append(halves)
            t = inpool.tile([128, scov], bf16, tag=f"ka{hh}", name=f"ka{hh}")
            for lo in range(0, scov, 512):
                hi = min(lo + 512, scov)
                nc.sync.dma_start(t[:, lo:hi], ka_d[hh][:, lo:hi])
            ka.append(t)
            halves = []
            for s2 in range(2):
                t = inpool.tile([128, L // 2], bf16, tag=f"qbc{hh}_{s2}",
                                name=f"qbc{hh}_{s2}")
                nc.sync.dma_start(t, qbc_d[hh][:, s2 * 1024:(s2 + 1) * 1024])
                halves.append(t)
            qbc.append(halves)
            t = inpool.tile([128, scov], bf16, tag=f"kbc{hh}", name=f"kbc{hh}")
            for lo in range(0, scov, 512):
                hi = min(lo + 512, scov)
                nc.sync.dma_start(t[:, lo:hi], kbc_d[hh][:, lo:hi])
            kbc.append(t)

        # va loads last: strided rearrange DMAs are descriptor-heavy and
        # not needed until the first phase2 group
        for hh in range(HPC):
            t = inpool.tile([128, schs[hh], D + 1], bf16, tag=f"va{hh}",
                            name=f"va{hh}")
            nc.sync.dma_start(t, va_d[hh][:, 0:schs[hh]])
            va.append(t)

        # identities built after the DMA issue so DGE starts immediately
        ident = consts.tile([128, 128], bf16)
        make_identity(nc, ident)
        ident32 = consts.tile([128, 128], f32)
        make_identity(nc, ident32)

        def qcols(tiles, lo, width):
            # slices never cross the 1024 boundary by construction
            return tiles[lo // 1024][:, lo % 1024:lo % 1024 + width]

        def phase1_group(hh, g):
            """scores + two-level top-32 extraction for tiles 4g..4g+3.

            Generator: yields after each fwd quarter so the emission can be
            interleaved with phase2 of the previous group (keeps the
            in-order PE stream free of dependency stalls)."""
            sch = schs[hh]
            scov = 128 * sch
            q4n = (sch + 3) // 4
            tcols = small.tile([128, 12], bf16, tag="tcols", name="tcols")
            for i in range(4):
                lt = 4 * g + i
                lhsA = qcols(qa[hh], lt * 128, 128)
                lhsBC = qcols(qbc[hh], lt * 128, 128)
                cand = candp.tile([128, 8 * sch], f32, tag=f"cand{hh}",
                                  name=f"cand{hh}")
                for q4 in range(q4n):
                    lo = q4 * 512
                    hi = min(lo + 512, scov)
                    pf = ps_f.tile([128, 512], f32, tag="fwd", name="fwd")
                    nc.tensor.matmul(pf[:, 0:hi - lo], lhsA, ka[hh][:, lo:hi],
                                     start=True, stop=False)
                    nc.tensor.matmul(pf[:, 0:hi - lo], lhsBC,
                                     kbc[hh][:, lo:hi], start=False, stop=True)
                    for c in range((hi - lo) // 128):
                        cc = 4 * q4 + c
                        nc.vector.max(out=cand[:, 8 * cc:8 * cc + 8],
                                      in_=pf[:, 128 * c:128 * (c + 1)])
                    if q4 < q4n - 1:
                        yield
                # chunk 8th-candidates (gathered before match_replace
                # destroys them) -> host overflow flags
                flgs = small.tile([128, 2 + sch], f32, tag=f"flgs{hh}",
                                  name=f"flgs{hh}")
                nc.gpsimd.tensor_copy(flgs[:, 2:2 + sch],
                                      cand[:, 7:8 * sch:8])
                # exact 32nd largest of the candidates
                m32 = small.tile([128, 32], f32, tag="m32", name="m32")
                for r in range(4):
                    nc.vector.max(out=m32[:, 8 * r:8 * r + 8], in_=cand)
                    nc.vector.match_replace(
                        out=cand, in_to_replace=m32[:, 8 * r:8 * r + 8],
                        in_values=cand, imm_value=NEG)
                # 33rd largest -> host margin-band flags
                m33 = small.tile([128, 8], f32, tag="m33", name="m33")
                nc.vector.max(out=m33, in_=cand)
                t32 = m32[:, 31:32]
                nc.gpsimd.tensor_copy(flgs[:, 0:1], t32)
                nc.gpsimd.tensor_copy(flgs[:, 1:2], m33[:, 0:1])
                nc.sync.dma_start(flg_d[hh, lt][:, 0:2 + sch], flgs)
                # m = -(t - |t|*2^-17 - 1e-30) = |t|*2^-17 + 1e-30 - t
                acol = small.tile([128, 4], f32, tag="tm", name="tm")
                nc.scalar.activation(acol[:, 0:1], t32,
                                     mybir.ActivationFunctionType.Abs,
                                     scale=float(2.0 ** -17))
                nc.vector.scalar_tensor_tensor(
                    out=acol[:, 1:2], in0=acol[:, 0:1], scalar=1e-30, in1=t32,
                    op0=mybir.AluOpType.add, op1=mybir.AluOpType.subtract)
                # bf16 triple split of m into tcols cols i, 4+i, 8+i
                nc.gpsimd.tensor_copy(tcols[:, i:i + 1], acol[:, 1:2])
                nc.gpsimd.tensor_tensor(
                    out=acol[:, 2:3], in0=acol[:, 1:2], in1=tcols[:, i:i + 1],
                    op=mybir.AluOpType.subtract)
                nc.gpsimd.tensor_copy(tcols[:, 4 + i:5 + i], acol[:, 2:3])
                nc.gpsimd.tensor_tensor(
                    out=acol[:, 3:4], in0=acol[:, 2:3],
                    in1=tcols[:, 4 + i:5 + i], op=mybir.AluOpType.subtract)
                nc.gpsimd.tensor_copy(tcols[:, 8 + i:9 + i], acol[:, 3:4])
            # transpose tcols into qa rows 65..67, cols of this q-group
            pt = ps_x.tile([128, 128], bf16, tag="tposeb", name="tposeb")
            nc.tensor.transpose(pt[0:12, :], tcols, ident)
            stage = small.tile([12, 128], bf16, tag="stage12", name="stage12")
            nc.scalar.copy(out=stage, in_=pt[0:12, :])
            half = qa[hh][g // 2]
            lo = (g * 512) % 1024
            for j in range(3):
                nc.sync.dma_start(
                    half[65 + j:66 + j, lo:lo + 512].rearrange(
                        "p (t q) -> p t q", t=4),
                    stage[4 * j:4 * (j + 1), :])
            yield

        def phase2_group(hh, g):
            sch = schs[hh]
            rhsA = qcols(qa[hh], g * 512, 512)
            rhsBC = qcols(qbc[hh], g * 512, 512)
            av = ps_av.tile([D + 1, 512], f32, tag="av", name="av")
            for c in range(sch):
                pt = ps_t.tile([128, 512], f32, tag="psumT", name="psumT")
                nc.tensor.matmul(pt, ka[hh][:, c * 128:(c + 1) * 128],
                                 rhsA, start=True, stop=False)
                nc.tensor.matmul(pt, kbc[hh][:, c * 128:(c + 1) * 128],
                                 rhsBC, start=False, stop=True)
                g_sb = gs_pool.tile([128, 512], bf16, tag="g", name="g")
                s_sb = gs_pool.tile([128, 512], bf16, tag="s", name="s")
                nc.scalar.activation(g_sb, pt,
                                     mybir.ActivationFunctionType.Exp,
                                     scale=float(TEMP))
                nc.scalar.activation(s_sb, pt,
                                     mybir.ActivationFunctionType.Sign)
                b2 = gs_pool.tile([128, 512], bf16, tag="b2", name="b2")
                nc.gpsimd.tensor_tensor(out=b2, in0=g_sb, in1=s_sb,
                                        op=mybir.AluOpType.mult)
                nc.tensor.matmul(av[0:D + 1], va[hh][:, c, :], g_sb,
                                 start=(c == 0), stop=False,
                                 skip_group_check=True)
                nc.tensor.matmul(av[0:D + 1], va[hh][:, c, :], b2,
                                 start=False, stop=(c == sch - 1),
                                 skip_group_check=True)
                if c < sch - 1:
                    yield
            # u = [2*sum_sel w*v ; 2Z]; normalize is deferred to the end
            # of the whole schedule to back-fill the tail of the pipeline
            u_sb = opool.tile([D + 1, 512], f32, tag="u", name="u")
            nc.scalar.copy(out=u_sb, in_=av)
            norm_list.append((hh, g, u_sb))
            yield

        def normalize_group(hh, g, u_sb):
            for sub in range(4):
                po = ps_t.tile([128, 512], f32, tag="psumT", name="psumT")
                nc.tensor.transpose(po[:, 0:D + 1],
                                    u_sb[:, sub * 128:(sub + 1) * 128],
                                    ident32[0:D + 1, 0:D + 1])
                recip = opool.tile([128, 1], f32, tag="recip", name="recip")
                nc.vector.reciprocal(out=recip, in_=po[:, D:D + 1])
                o_sb = opool.tile([128, D], f32, tag="osb", name="osb")
                nc.vector.tensor_scalar(
                    out=o_sb, in0=po[:, 0:D], scalar1=recip, scalar2=None,
                    op0=mybir.AluOpType.mult)
                lq = g * 512 + sub * 128
                nc.sync.dma_start(out_d[hh, lq:lq + 128, :], o_sb)
                yield

        def drive(p2gen, p1gen, ratio=1):
            """Round-robin the two emission generators until exhausted,
            advancing the partner `ratio` steps per p2 step."""
            alive = [p2gen, p1gen]
            while any(alive):
                for i, gen in enumerate(alive):
                    for _ in range(1 if i == 0 else ratio):
                        if alive[i] is not None:
                            try:
                                next(alive[i])
                            except StopIteration:
                                alive[i] = None

        norm_list = []

        def norms_upto(n):
            for item in norm_list[:n]:
                yield from normalize_group(*item)

        groups = [(hh, g) for hh in range(HPC) for g in range(QB)]
        drive(None, phase1_group(*groups[0]))
        for k in range(len(groups) - 1):
            drive(phase2_group(*groups[k]), phase1_group(*groups[k + 1]))
        # last group's chunks interleave with all deferred normalizes
        drive(phase2_group(*groups[-1]), norms_upto(len(groups) - 1),
              ratio=3)
        for item in norm_list[len(groups) - 1:]:
            for _ in normalize_group(*item):
                pass

    nc.compile()
    return nc


_NC_CACHE = {}


def _get_nc(schs):
    key = tuple(schs)
    if key not in _NC_CACHE:
        _NC_CACHE[key] = _build_bass(key)
    return _NC_CACHE[key]


def _split_hi_lo(x):
    hi = x.astype(_bf16)
    lo = (x.astype(np.float32) - hi.astype(np.float32)).astype(_bf16)
    return hi, lo


def _host_fix_rows(out, rows, queries, keys, values, key_lengths):
    """Exact fp32 recompute of rows whose on-device selection count != 32."""
    for (n, lq, h) in rows:
        q = np.asarray(queries[n, lq, h, :], np.float32)
        K = np.asarray(keys[n, :, h, :], np.float32)
        V = np.asarray(values[n, :, h, :], np.float32)
        kl = int(key_lengths[n])
        s = K @ q
        s[kl:] = -np.inf
        idx = np.argsort(-s, kind="stable")[:TOPK]
        w = np.exp(TEMP * (s[idx] - s[idx].max()))
        out[n, lq, h, :] = (w[:, None] * V[idx]).sum(0) / w.sum()


def _prep_core(core, queries, keys, values, key_lengths_i, schs):
    # one head of each batch per core: per-head valid-chunk counts
    pairs = [(0, core), (1, core)]
    schmax = max(schs)
    qa = np.zeros((HEADS_PER_CORE, 128, L), _bf16)
    ka = np.zeros((HEADS_PER_CORE, 128, S), _bf16)
    qbc = np.zeros((HEADS_PER_CORE, 128, L), _bf16)
    kbc = np.zeros((HEADS_PER_CORE, 128, S), _bf16)
    va = np.zeros((HEADS_PER_CORE, 128, schmax, D + 1), _bf16)
    for i, (n, h) in enumerate(pairs):
        sch = schs[i]
        scov = 128 * sch
        Q = queries[n, :, h, :]           # [L, E]
        K = keys[n, :, h, :]              # [S, E]
        V = values[n, :, h, :]            # [S, D]
        qh, ql = _split_hi_lo(Q)
        kh, kl_ = _split_hi_lo(K)
        mask = np.where(np.arange(S) < int(key_lengths_i[n]), 0.0, NEG
                        ).astype(np.float32)
        qa[i, 0:E, :] = qh.T
        qa[i, E, :] = 1.0
        # rows 65..67 stay 0 (t slots, filled on device)
        qa[i, E + 4:E + 4 + NLO, :] = ql.T[0:NLO]
        ka[i, 0:E, :] = kh.T
        ka[i, E, :] = mask.astype(_bf16)
        ka[i, E + 1:E + 4, :] = 1.0
        ka[i, E + 4:E + 4 + NLO, :] = kl_.T[0:NLO]
        qbc[i, 0:E, :] = qh.T
        qbc[i, E:2 * E, :] = ql.T
        kbc[i, 0:E, :] = kl_.T
        kbc[i, E:2 * E, :] = kh.T
        va[i, :, 0:sch, 0:D] = np.moveaxis(
            V.astype(_bf16)[0:scov].reshape(sch, 128, D), 0, 1)
        va[i, :, 0:sch, D] = 1.0
    return pairs, {"qa": qa, "ka": ka, "qbc": qbc, "kbc": kbc, "va": va}


def kernel(queries, keys, values, key_lengths):
    from concourse.bass_utils import run_bass_kernel_spmd

    queries = np.asarray(queries, np.float32)
    keys = np.asarray(keys, np.float32)
    values = np.asarray(values, np.float32)
    key_lengths_i = np.asarray(key_lengths).astype(np.int64)
    schs = tuple(int(min((int(key_lengths_i[n]) + 127) // 128, S // 128))
                 for n in range(N))

    in_maps = []
    head_map = []  # per core: list of (n, h)
    for core in range(N_CORES):
        pairs, im = _prep_core(core, queries, keys, values, key_lengths_i,
                               schs)
        head_map.append(pairs)
        in_maps.append(im)

    nc = _get_nc(schs)
    res = run_bass_kernel_spmd(nc, in_maps, list(range(N_CORES)))

    schmax = max(schs)
    out = np.zeros((N, L, H, D), np.float32)
    fix_rows = []
    for core in range(N_CORES):
        o = res.results[core]["out"].reshape(HEADS_PER_CORE, L, D)
        flg = res.results[core]["flg"].reshape(HEADS_PER_CORE, L, 2 + schmax)
        for i, (n, h) in enumerate(head_map[core]):
            out[n, :, h, :] = o[i]
            for lq in np.nonzero(_flag_rows(flg[i], schs[i]))[0]:
                fix_rows.append((n, int(lq), h))
    if fix_rows:
        _host_fix_rows(out, fix_rows, queries, keys, values, key_lengths_i)
    return out


def _flag_rows(flg, sch):
    """flg: [L, 2+>=sch] = [t_hat, cand33, chunk 8ths...] -> bool [L]."""
    t_hat = flg[:, 0]
    c33 = flg[:, 1]
    ch8 = flg[:, 2:2 + sch]
    overflow = (ch8 >= t_hat[:, None]).any(axis=1)
    margin = c33 >= t_hat - np.abs(t_hat) * 2.0 ** -16
    return overflow | margin
